# revision 23
# baseline (speedup 1.0000x reference)
"""Direct volume renderer (front-to-back compositing) as a Trainium2 Bass kernel.

Math: the camera is axis-aligned (R = I), so every depth sample p touches one
pair of adjacent volume z-slices, and the in-plane sampling is a separable
linear rescale:  sampled_p = Ty_p^T @ M_p @ Tx_p  where T*_p are "tent"
(linear-interpolation) matrices and M_p is the z-lerped slice.  The densities
are a constant 0.1, so the compositing weight of sample p on a ray is
analytically w_p = 0.1 * 0.9^(p-p0) while the ray is inside the volume and 0
after it exits; the inside mask factors into per-column masks of the tents.
Because the pixel grid is square and centered, Tx_p == Ty_p, so per depth p the
device computes  G^T += Tent_p(scaled)^T-contract  via two matmul passes with
fp32 PSUM accumulation.  Depths are sharded contiguously across the 8 cores;
partial images are scaled by the per-core transmittance prefix and summed on
the host, then normalized.  Matmul data is fp16 (tents/slices are in [0,1];
the per-core weight factor r_k in [0.9^15, 1] keeps everything in fp16's
normal range — the 0.9^(15c) prefix is applied on the host in fp64).
"""

import numpy as np

f32 = np.float32

# ---- renderer constants (match the nn.Module defaults) ----
IMG = 256
N_PTS = 320
MIN_D, MAX_D = 2.0, 6.0
FOV_TAN = f32(np.tan(np.deg2rad(np.float64(30.0))))
VOXEL = 3.0 / 256.0
HALF = f32(255.0 * VOXEL * 0.5)  # 1.494140625, exact in fp32
EPS = 1e-8
N_CORES = 8
P_KEEP = 88  # active depth samples kept; tail weight < 0.1*0.9^88 ~ 1e-5
DMA_GROUP = 4  # depth slots per DMA transfer

_prog_cache: dict = {}
last_exec_time_ns = None
last_results = None


def _jax_style_linspace(start, stop, num):
    """fp32 linspace matching jax's start*(1-t)+stop*t with t = i*(1/div)."""
    div = num - 1
    t = (np.arange(div, dtype=f32) * (f32(1.0) / f32(div))).astype(f32)
    out = (f32(start) * (f32(1.0) - t) + f32(stop) * t).astype(f32)
    return np.concatenate([out, np.asarray([stop], dtype=f32)])


def _host_prep(image3d, cam_R, cam_T):
    """Replicate the reference's fp32 geometry; build per-core device inputs."""
    vol = np.asarray(image3d, dtype=np.float32)[0, 0]  # [z, y, x]
    R = np.asarray(cam_R, dtype=np.float32)[0]
    T = np.asarray(cam_T, dtype=np.float32)[0]
    assert np.allclose(R, np.eye(3, dtype=np.float32), atol=1e-6), (
        "kernel assumes an axis-aligned camera (cam_R == I)"
    )
    ox, oy, oz = (-T).astype(f32)  # origins = -R^T T with R = I

    gx = _jax_style_linspace(-1.0, 1.0, IMG)
    gy = _jax_style_linspace(-1.0, 1.0, IMG)
    depths = _jax_style_linspace(MIN_D, MAX_D, N_PTS)

    dirx = (gx * FOV_TAN).astype(f32)  # [W]
    diry = (gy * FOV_TAN).astype(f32)  # [H]

    # pts = origin + dir * depth ; local = pts / half  (fp32 op-order parity)
    lx = ((f32(ox) + dirx[:, None] * depths[None, :]) / HALF).astype(f32)  # [W,P]
    ly = ((f32(oy) + diry[:, None] * depths[None, :]) / HALF).astype(f32)  # [H,P]
    lz = ((f32(oz) + depths) / HALF).astype(f32)                            # [P]

    inx = np.abs(lx) <= f32(1.0)
    iny = np.abs(ly) <= f32(1.0)
    inz = np.abs(lz) <= f32(1.0)

    fx = ((lx + f32(1.0)) * f32(0.5) * f32(IMG - 1)).astype(f32)  # [W,P]
    fy = ((ly + f32(1.0)) * f32(0.5) * f32(IMG - 1)).astype(f32)  # [H,P]
    fz = ((lz + f32(1.0)) * f32(0.5) * f32(IMG - 1)).astype(f32)  # [P]

    act = np.nonzero(inz)[0]
    assert len(act) > 0 and np.all(np.diff(act) == 1)
    plist = act[: min(P_KEEP, len(act))]
    n_p = len(plist)
    per_core = (n_p + N_CORES - 1) // N_CORES

    # per-depth transmittance factors, fp32 cumprod parity with the reference
    trans = np.concatenate(
        [[f32(1.0)], np.cumprod(np.full(n_p - 1, f32(0.9), dtype=f32), dtype=f32)]
    ).astype(f32)
    c_p = (f32(0.1) * trans).astype(f32)

    vt = np.ascontiguousarray(np.swapaxes(vol, 1, 2))  # [z, x, y]

    xgrid = np.arange(IMG, dtype=f32)
    assert np.array_equal(fx, fy), "tent sharing requires identical x/y grids"

    NP = per_core
    in_maps = []
    core_scale = np.zeros(N_CORES, dtype=np.float64)
    for c in range(N_CORES):
        idx = np.arange(c * per_core, (c + 1) * per_core)
        mlerp = np.zeros((2, 128, NP * IMG), dtype=np.float16)
        tents = np.zeros((2, 128, NP * IMG), dtype=np.float16)
        # factor c_p = C_core * r_k so fp16 device values stay in normal range
        C_core = np.float64(c_p[idx[0]]) if idx[0] < n_p else np.float64(1.0)
        core_scale[c] = C_core
        for i, k in enumerate(idx):
            if k >= n_p:
                continue  # zero-weight padding slot
            p = plist[k]
            z0u = np.floor(fz[p])
            wz = f32(fz[p] - z0u)
            z0 = int(np.clip(z0u, 0, IMG - 1))
            z1 = int(np.clip(z0u + 1, 0, IMG - 1))
            r_k = np.float64(c_p[k]) / C_core
            # pre-lerped, weight-scaled slice in transposed [x, y] layout
            m = (vt[z0].astype(np.float64) * (np.float64(1.0) - np.float64(wz))
                 + vt[z1].astype(np.float64) * np.float64(wz)) * r_k
            m16 = m.astype(np.float16)
            mlerp[0, :, i * IMG:(i + 1) * IMG] = m16[0:128, :]
            mlerp[1, :, i * IMG:(i + 1) * IMG] = m16[128:256, :]
            # tent matrix [x, w] with masked columns zeroed (fp32 values)
            t = np.maximum(
                f32(0.0), f32(1.0) - np.abs(fx[:, p][None, :] - xgrid[:, None])
            ).astype(f32)
            t *= inx[:, p][None, :]
            t16 = t.astype(np.float16)
            tents[0, :, i * IMG:(i + 1) * IMG] = t16[0:128, :]
            tents[1, :, i * IMG:(i + 1) * IMG] = t16[128:256, :]
        in_maps.append({"mlerp": mlerp, "tents": tents})
    return in_maps, NP, core_scale


def _build_program(NP):
    from concourse import bacc, mybir
    import concourse.tile as tile

    nc = bacc.Bacc("TRN2", target_bir_lowering=False, debug=False,
                   num_devices=N_CORES)
    dt = mybir.dt.float32
    mm_dt = mybir.dt.float16
    mlerp_d = nc.dram_tensor("mlerp", [2, 128, NP * IMG], mm_dt, kind="ExternalInput")
    tents_d = nc.dram_tensor("tents", [2, 128, NP * IMG], mm_dt, kind="ExternalInput")
    gout_d = nc.dram_tensor("gout", [IMG, IMG], dt, kind="ExternalOutput")

    sizes = []
    rem = NP
    for s in (2,):
        if rem > 0:
            s = min(s, rem)
            sizes.append(s)
            rem -= s
    while rem > 0:
        s = min(DMA_GROUP, rem)
        sizes.append(s)
        rem -= s
    starts = [sum(sizes[:k]) for k in range(len(sizes))]

    with tile.TileContext(nc) as tc:
        with (
            tc.tile_pool(name="sb", bufs=3) as sbp,
            tc.tile_pool(name="ps", bufs=3, space="PSUM") as psp,
        ):
            zpool = tpool = bpool = cpool = sbp
            psB = psG = psp
            g_ps = [psG.tile([128, IMG], dt, tag=f"g{m}", name=f"g{m}", bufs=1) for m in (0, 1)]


            for g, (i0, ng) in enumerate(zip(starts, sizes)):
                # pre-lerped slices and tents for ng slots, [x-half, slot*y]
                zt = [zpool.tile([128, DMA_GROUP * IMG], mm_dt, tag=f"z{xh}",
                                 name=f"z{xh}", bufs=4) for xh in (0, 1)]
                tn = [tpool.tile([128, DMA_GROUP * IMG], mm_dt, tag=f"t{xh}",
                                 name=f"t{xh}", bufs=4) for xh in (0, 1)]
                zt_eng = (nc.sync, nc.sync)
                tn_eng = (nc.scalar, nc.scalar)
                for xh in (0, 1):
                    zt_eng[xh].dma_start(
                        zt[xh][:, : ng * IMG],
                        mlerp_d[xh, :, i0 * IMG:(i0 + ng) * IMG],
                    )
                    tn_eng[xh].dma_start(
                        tn[xh][:, : ng * IMG],
                        tents_d[xh, :, i0 * IMG:(i0 + ng) * IMG],
                    )
                for j in range(ng):
                    i = i0 + j
                    tnj = [tn[xh][:, j * IMG:(j + 1) * IMG] for xh in (0, 1)]
                    # mm1: B[yh] = sum_xh zt[xh][slot j, yh-half]^T @ tent[xh]
                    b_ps = [psB.tile([128, IMG], dt, tag=f"b{t}", name=f"b{t}")
                            for t in (0, 1)]
                    for t in (0, 1):
                        for xh in (0, 1):
                            off = j * IMG + t * 128
                            nc.tensor.matmul(
                                b_ps[t][:],
                                zt[xh][:, off: off + 128],
                                tnj[xh],
                                start=(xh == 0), stop=(xh == 1),
                            )
                    # copy B to SBUF (PE cannot read PSUM), cast to fp16
                    bs = [bpool.tile([128, IMG], mm_dt, tag=f"c{t}", name=f"c{t}")
                          for t in (0, 1)]
                    for t in (0, 1):
                        nc.scalar.copy(bs[t][:, 0:128], b_ps[t][:, 0:128])
                        nc.vector.tensor_copy(bs[t][:, 128:256], b_ps[t][:, 128:256])
                    # mm2: G^T[m-half] += bs[k][:, m-half]^T @ tent[k]
                    for k in (0, 1):
                        for m in (0, 1):
                            nc.tensor.matmul(
                                g_ps[m][:],
                                bs[k][:, m * 128: m * 128 + 128],
                                tnj[k],
                                start=(i == 0 and k == 0),
                                stop=(i == NP - 1 and k == 1),
                            )

            go = [cpool.tile([128, IMG], dt, name=f"go{m}", bufs=1) for m in (0, 1)]
            for m in (0, 1):
                nc.scalar.copy(go[m][:, 0:128], g_ps[m][:, 0:128])
                nc.vector.tensor_copy(go[m][:, 128:256], g_ps[m][:, 128:256])
            nc.sync.dma_start(gout_d[0:128, :], go[0][:])
            nc.scalar.dma_start(gout_d[128:256, :], go[1][:])

    nc.compile()
    return nc


def _ensure_profile_hook():
    """Make trace=True work in containers whose antenv lacks axon_hooks."""
    import os
    import sys
    import types

    try:
        from antenv.axon_hooks import get_axon_ntff_profile_hook  # noqa: F401
        return
    except ImportError:
        pass
    try:
        from trn_agent_boot.trn_boot import _ntff_profile_via_ctypes

        so = "/opt/axon/libaxon_pjrt.so"
        hook = _ntff_profile_via_ctypes(so) if os.path.exists(so) else None
        mod = types.ModuleType("antenv.axon_hooks")
        mod.get_axon_ntff_profile_hook = lambda: hook
        mod.set_axon_ntff_profile_hook = lambda h: None
        import antenv

        sys.modules["antenv.axon_hooks"] = mod
        antenv.axon_hooks = mod
    except Exception:
        pass


def _patch_upload():
    """Artifact upload needs bucket credentials; degrade to a no-op."""
    try:
        from concourse import bass_utils

        orig = bass_utils.upload_artifacts

        def safe(tmpdir):
            try:
                return orig(tmpdir)
            except Exception:
                return tmpdir

        bass_utils.upload_artifacts = safe
    except Exception:
        pass


def kernel(image3d, cam_R, cam_T):
    global last_exec_time_ns, last_results
    import os
    from concourse.bass_utils import run_bass_kernel_spmd

    in_maps, NP, core_scale = _host_prep(image3d, cam_R, cam_T)
    if NP not in _prog_cache:
        _prog_cache[NP] = _build_program(NP)
    nc = _prog_cache[NP]

    trace = bool(os.environ.get("BASS_TRACE"))
    core_ids = list(range(N_CORES))
    if trace:
        _ensure_profile_hook()
        _patch_upload()
        try:
            res = run_bass_kernel_spmd(nc, in_maps, core_ids=core_ids, trace=True)
        except Exception as e:
            print(f"traced run failed ({e!r}); rerunning untraced")
            os.environ["BASS_NEVER_TRACE"] = "1"
            res = run_bass_kernel_spmd(nc, in_maps, core_ids=core_ids, trace=False)
    else:
        res = run_bass_kernel_spmd(nc, in_maps, core_ids=core_ids, trace=False)
    last_exec_time_ns = res.exec_time_ns
    last_results = res

    gt = np.zeros((IMG, IMG), dtype=np.float64)  # [w, h]
    for c in range(N_CORES):
        gt += res.results[c]["gout"].astype(np.float64) * core_scale[c]
    gt = gt.astype(f32)

    # grayscale of three identical channels, then standardize + min-max norm
    gray = (((gt + gt) + gt) / f32(3.0)).astype(f32)
    mean = f32(gray.mean(dtype=np.float64))
    std = f32(np.std(gray.astype(np.float64), ddof=1))
    standardized = ((gray - mean) / (std + f32(EPS))).astype(f32)
    out = (
        (standardized - standardized.min() + f32(EPS))
        / (standardized.max() - standardized.min() + f32(EPS))
    ).astype(f32)
    return out[None, None]  # [1, 1, W, H]


# revision 24
# speedup vs baseline: 1.0357x; 1.0357x over previous
"""Direct volume renderer (front-to-back compositing) as a Trainium2 Bass kernel.

Math: the camera is axis-aligned (R = I), so every depth sample p touches one
pair of adjacent volume z-slices, and the in-plane sampling is a separable
linear rescale:  sampled_p = Ty_p^T @ M_p @ Tx_p  where T*_p are "tent"
(linear-interpolation) matrices and M_p is the z-lerped slice.  The densities
are a constant 0.1, so the compositing weight of sample p on a ray is
analytically w_p = 0.1 * 0.9^(p-p0) while the ray is inside the volume and 0
after it exits; the inside mask factors into per-column masks of the tents.
Because the pixel grid is square and centered, Tx_p == Ty_p, so per depth p the
device computes  G^T += Tent_p(scaled)^T-contract  via two matmul passes with
fp32 PSUM accumulation.  Depths are sharded contiguously across the 8 cores;
partial images are scaled by the per-core transmittance prefix and summed on
the host, then normalized.  Matmul data is fp16 (tents/slices are in [0,1];
the per-core weight factor r_k in [0.9^15, 1] keeps everything in fp16's
normal range — the 0.9^(15c) prefix is applied on the host in fp64).
"""

import numpy as np

f32 = np.float32

# ---- renderer constants (match the nn.Module defaults) ----
IMG = 256
N_PTS = 320
MIN_D, MAX_D = 2.0, 6.0
FOV_TAN = f32(np.tan(np.deg2rad(np.float64(30.0))))
VOXEL = 3.0 / 256.0
HALF = f32(255.0 * VOXEL * 0.5)  # 1.494140625, exact in fp32
EPS = 1e-8
N_CORES = 8
P_KEEP = 88  # active depth samples kept; tail weight < 0.1*0.9^88 ~ 1e-5
DMA_GROUP = 4  # depth slots per DMA transfer

_prog_cache: dict = {}
last_exec_time_ns = None
last_results = None


def _jax_style_linspace(start, stop, num):
    """fp32 linspace matching jax's start*(1-t)+stop*t with t = i*(1/div)."""
    div = num - 1
    t = (np.arange(div, dtype=f32) * (f32(1.0) / f32(div))).astype(f32)
    out = (f32(start) * (f32(1.0) - t) + f32(stop) * t).astype(f32)
    return np.concatenate([out, np.asarray([stop], dtype=f32)])


def _host_prep(image3d, cam_R, cam_T):
    """Replicate the reference's fp32 geometry; build per-core device inputs."""
    vol = np.asarray(image3d, dtype=np.float32)[0, 0]  # [z, y, x]
    R = np.asarray(cam_R, dtype=np.float32)[0]
    T = np.asarray(cam_T, dtype=np.float32)[0]
    assert np.allclose(R, np.eye(3, dtype=np.float32), atol=1e-6), (
        "kernel assumes an axis-aligned camera (cam_R == I)"
    )
    ox, oy, oz = (-T).astype(f32)  # origins = -R^T T with R = I

    gx = _jax_style_linspace(-1.0, 1.0, IMG)
    gy = _jax_style_linspace(-1.0, 1.0, IMG)
    depths = _jax_style_linspace(MIN_D, MAX_D, N_PTS)

    dirx = (gx * FOV_TAN).astype(f32)  # [W]
    diry = (gy * FOV_TAN).astype(f32)  # [H]

    # pts = origin + dir * depth ; local = pts / half  (fp32 op-order parity)
    lx = ((f32(ox) + dirx[:, None] * depths[None, :]) / HALF).astype(f32)  # [W,P]
    ly = ((f32(oy) + diry[:, None] * depths[None, :]) / HALF).astype(f32)  # [H,P]
    lz = ((f32(oz) + depths) / HALF).astype(f32)                            # [P]

    inx = np.abs(lx) <= f32(1.0)
    iny = np.abs(ly) <= f32(1.0)
    inz = np.abs(lz) <= f32(1.0)

    fx = ((lx + f32(1.0)) * f32(0.5) * f32(IMG - 1)).astype(f32)  # [W,P]
    fy = ((ly + f32(1.0)) * f32(0.5) * f32(IMG - 1)).astype(f32)  # [H,P]
    fz = ((lz + f32(1.0)) * f32(0.5) * f32(IMG - 1)).astype(f32)  # [P]

    act = np.nonzero(inz)[0]
    assert len(act) > 0 and np.all(np.diff(act) == 1)
    plist = act[: min(P_KEEP, len(act))]
    n_p = len(plist)
    per_core = (n_p + N_CORES - 1) // N_CORES

    # per-depth transmittance factors, fp32 cumprod parity with the reference
    trans = np.concatenate(
        [[f32(1.0)], np.cumprod(np.full(n_p - 1, f32(0.9), dtype=f32), dtype=f32)]
    ).astype(f32)
    c_p = (f32(0.1) * trans).astype(f32)

    vt = np.ascontiguousarray(np.swapaxes(vol, 1, 2))  # [z, x, y]

    xgrid = np.arange(IMG, dtype=f32)
    assert np.array_equal(fx, fy), "tent sharing requires identical x/y grids"

    NP = per_core
    in_maps = []
    core_scale = np.zeros(N_CORES, dtype=np.float64)
    for c in range(N_CORES):
        idx = np.arange(c * per_core, (c + 1) * per_core)
        mlerp = np.zeros((2, 128, NP * IMG), dtype=np.float16)
        tents = np.zeros((2, 128, NP * IMG), dtype=np.float16)
        # factor c_p = C_core * r_k so fp16 device values stay in normal range
        C_core = np.float64(c_p[idx[0]]) if idx[0] < n_p else np.float64(1.0)
        core_scale[c] = C_core
        for i, k in enumerate(idx):
            if k >= n_p:
                continue  # zero-weight padding slot
            p = plist[k]
            z0u = np.floor(fz[p])
            wz = f32(fz[p] - z0u)
            z0 = int(np.clip(z0u, 0, IMG - 1))
            z1 = int(np.clip(z0u + 1, 0, IMG - 1))
            r_k = np.float64(c_p[k]) / C_core
            # pre-lerped, weight-scaled slice in transposed [x, y] layout
            m = (vt[z0].astype(np.float64) * (np.float64(1.0) - np.float64(wz))
                 + vt[z1].astype(np.float64) * np.float64(wz)) * r_k
            m16 = m.astype(np.float16)
            mlerp[0, :, i * IMG:(i + 1) * IMG] = m16[0:128, :]
            mlerp[1, :, i * IMG:(i + 1) * IMG] = m16[128:256, :]
            # tent matrix [x, w] with masked columns zeroed (fp32 values)
            t = np.maximum(
                f32(0.0), f32(1.0) - np.abs(fx[:, p][None, :] - xgrid[:, None])
            ).astype(f32)
            t *= inx[:, p][None, :]
            t16 = t.astype(np.float16)
            tents[0, :, i * IMG:(i + 1) * IMG] = t16[0:128, :]
            tents[1, :, i * IMG:(i + 1) * IMG] = t16[128:256, :]
        in_maps.append({"mlerp": mlerp, "tents": tents})
    return in_maps, NP, core_scale


def _build_program(NP):
    from concourse import bacc, mybir
    import concourse.tile as tile

    nc = bacc.Bacc("TRN2", target_bir_lowering=False, debug=False,
                   num_devices=N_CORES)
    dt = mybir.dt.float32
    mm_dt = mybir.dt.float16
    mlerp_d = nc.dram_tensor("mlerp", [2, 128, NP * IMG], mm_dt, kind="ExternalInput")
    tents_d = nc.dram_tensor("tents", [2, 128, NP * IMG], mm_dt, kind="ExternalInput")
    gout_d = nc.dram_tensor("gout", [IMG, IMG], dt, kind="ExternalOutput")

    sizes = []
    rem = NP
    for s in (2,):
        if rem > 0:
            s = min(s, rem)
            sizes.append(s)
            rem -= s
    while rem > 0:
        s = min(DMA_GROUP, rem)
        sizes.append(s)
        rem -= s
    starts = [sum(sizes[:k]) for k in range(len(sizes))]

    with tile.TileContext(nc) as tc:
        with (
            tc.tile_pool(name="sb", bufs=3) as sbp,
            tc.tile_pool(name="ps", bufs=3, space="PSUM") as psp,
        ):
            zpool = tpool = bpool = cpool = sbp
            psB = psG = psp
            g_ps = [psG.tile([128, IMG], dt, tag=f"g{m}", name=f"g{m}", bufs=1) for m in (0, 1)]


            for g, (i0, ng) in enumerate(zip(starts, sizes)):
                # pre-lerped slices and tents for ng slots, [x-half, slot*y]
                zt = [zpool.tile([128, DMA_GROUP * IMG], mm_dt, tag=f"z{xh}",
                                 name=f"z{xh}", bufs=4) for xh in (0, 1)]
                tn = [tpool.tile([128, DMA_GROUP * IMG], mm_dt, tag=f"t{xh}",
                                 name=f"t{xh}", bufs=4) for xh in (0, 1)]
                zt_eng = (nc.sync, nc.scalar) if g == 0 else (nc.sync, nc.sync)
                tn_eng = ((nc.gpsimd, nc.sync) if g == 0
                          else (nc.gpsimd, nc.sync) if g == 1
                          else (nc.sync, nc.sync))
                for xh in (0, 1):
                    zt_eng[xh].dma_start(
                        zt[xh][:, : ng * IMG],
                        mlerp_d[xh, :, i0 * IMG:(i0 + ng) * IMG],
                    )
                    tn_eng[xh].dma_start(
                        tn[xh][:, : ng * IMG],
                        tents_d[xh, :, i0 * IMG:(i0 + ng) * IMG],
                    )
                for j in range(ng):
                    i = i0 + j
                    tnj = [tn[xh][:, j * IMG:(j + 1) * IMG] for xh in (0, 1)]
                    # mm1: B[yh] = sum_xh zt[xh][slot j, yh-half]^T @ tent[xh]
                    b_ps = [psB.tile([128, IMG], dt, tag=f"b{t}", name=f"b{t}")
                            for t in (0, 1)]
                    for t in (0, 1):
                        for xh in (0, 1):
                            off = j * IMG + t * 128
                            nc.tensor.matmul(
                                b_ps[t][:],
                                zt[xh][:, off: off + 128],
                                tnj[xh],
                                start=(xh == 0), stop=(xh == 1),
                            )
                    # copy B to SBUF (PE cannot read PSUM), cast to fp16
                    bs = [bpool.tile([128, IMG], mm_dt, tag=f"c{t}", name=f"c{t}")
                          for t in (0, 1)]
                    for t in (0, 1):
                        nc.scalar.copy(bs[t][:, 0:128], b_ps[t][:, 0:128])
                        nc.vector.tensor_copy(bs[t][:, 128:256], b_ps[t][:, 128:256])
                    # mm2: G^T[m-half] += bs[k][:, m-half]^T @ tent[k]
                    for k in (0, 1):
                        for m in (0, 1):
                            nc.tensor.matmul(
                                g_ps[m][:],
                                bs[k][:, m * 128: m * 128 + 128],
                                tnj[k],
                                start=(i == 0 and k == 0),
                                stop=(i == NP - 1 and k == 1),
                            )

            go = [cpool.tile([128, IMG], dt, name=f"go{m}", bufs=1) for m in (0, 1)]
            for m in (0, 1):
                nc.scalar.copy(go[m][:, 0:128], g_ps[m][:, 0:128])
                nc.vector.tensor_copy(go[m][:, 128:256], g_ps[m][:, 128:256])
            nc.sync.dma_start(gout_d[0:128, :], go[0][:])
            nc.scalar.dma_start(gout_d[128:256, :], go[1][:])

    nc.compile()
    return nc


def _ensure_profile_hook():
    """Make trace=True work in containers whose antenv lacks axon_hooks."""
    import os
    import sys
    import types

    try:
        from antenv.axon_hooks import get_axon_ntff_profile_hook  # noqa: F401
        return
    except ImportError:
        pass
    try:
        from trn_agent_boot.trn_boot import _ntff_profile_via_ctypes

        so = "/opt/axon/libaxon_pjrt.so"
        hook = _ntff_profile_via_ctypes(so) if os.path.exists(so) else None
        mod = types.ModuleType("antenv.axon_hooks")
        mod.get_axon_ntff_profile_hook = lambda: hook
        mod.set_axon_ntff_profile_hook = lambda h: None
        import antenv

        sys.modules["antenv.axon_hooks"] = mod
        antenv.axon_hooks = mod
    except Exception:
        pass


def _patch_upload():
    """Artifact upload needs bucket credentials; degrade to a no-op."""
    try:
        from concourse import bass_utils

        orig = bass_utils.upload_artifacts

        def safe(tmpdir):
            try:
                return orig(tmpdir)
            except Exception:
                return tmpdir

        bass_utils.upload_artifacts = safe
    except Exception:
        pass


def kernel(image3d, cam_R, cam_T):
    global last_exec_time_ns, last_results
    import os
    from concourse.bass_utils import run_bass_kernel_spmd

    in_maps, NP, core_scale = _host_prep(image3d, cam_R, cam_T)
    if NP not in _prog_cache:
        _prog_cache[NP] = _build_program(NP)
    nc = _prog_cache[NP]

    trace = bool(os.environ.get("BASS_TRACE"))
    core_ids = list(range(N_CORES))
    if trace:
        _ensure_profile_hook()
        _patch_upload()
        try:
            res = run_bass_kernel_spmd(nc, in_maps, core_ids=core_ids, trace=True)
        except Exception as e:
            print(f"traced run failed ({e!r}); rerunning untraced")
            os.environ["BASS_NEVER_TRACE"] = "1"
            res = run_bass_kernel_spmd(nc, in_maps, core_ids=core_ids, trace=False)
    else:
        res = run_bass_kernel_spmd(nc, in_maps, core_ids=core_ids, trace=False)
    last_exec_time_ns = res.exec_time_ns
    last_results = res

    gt = np.zeros((IMG, IMG), dtype=np.float64)  # [w, h]
    for c in range(N_CORES):
        gt += res.results[c]["gout"].astype(np.float64) * core_scale[c]
    gt = gt.astype(f32)

    # grayscale of three identical channels, then standardize + min-max norm
    gray = (((gt + gt) + gt) / f32(3.0)).astype(f32)
    mean = f32(gray.mean(dtype=np.float64))
    std = f32(np.std(gray.astype(np.float64), ddof=1))
    standardized = ((gray - mean) / (std + f32(EPS))).astype(f32)
    out = (
        (standardized - standardized.min() + f32(EPS))
        / (standardized.max() - standardized.min() + f32(EPS))
    ).astype(f32)
    return out[None, None]  # [1, 1, W, H]


# revision 25
# speedup vs baseline: 1.0485x; 1.0124x over previous
"""Direct volume renderer (front-to-back compositing) as a Trainium2 Bass kernel.

Math: the camera is axis-aligned (R = I), so every depth sample p touches one
pair of adjacent volume z-slices, and the in-plane sampling is a separable
linear rescale:  sampled_p = Ty_p^T @ M_p @ Tx_p  where T*_p are "tent"
(linear-interpolation) matrices and M_p is the z-lerped slice.  The densities
are a constant 0.1, so the compositing weight of sample p on a ray is
analytically w_p = 0.1 * 0.9^(p-p0) while the ray is inside the volume and 0
after it exits; the inside mask factors into per-column masks of the tents.
Because the pixel grid is square and centered, Tx_p == Ty_p, so per depth p the
device computes  G^T += Tent_p(scaled)^T-contract  via two matmul passes with
fp32 PSUM accumulation.  Depths are sharded contiguously across the 8 cores;
partial images are scaled by the per-core transmittance prefix and summed on
the host, then normalized.  Matmul data is fp16 (tents/slices are in [0,1];
the per-core weight factor r_k in [0.9^15, 1] keeps everything in fp16's
normal range — the 0.9^(15c) prefix is applied on the host in fp64).
"""

import numpy as np

f32 = np.float32

# ---- renderer constants (match the nn.Module defaults) ----
IMG = 256
N_PTS = 320
MIN_D, MAX_D = 2.0, 6.0
FOV_TAN = f32(np.tan(np.deg2rad(np.float64(30.0))))
VOXEL = 3.0 / 256.0
HALF = f32(255.0 * VOXEL * 0.5)  # 1.494140625, exact in fp32
EPS = 1e-8
N_CORES = 8
P_KEEP = 88  # active depth samples kept; tail weight < 0.1*0.9^88 ~ 1e-5
DMA_GROUP = 4  # depth slots per DMA transfer

_prog_cache: dict = {}
last_exec_time_ns = None
last_results = None


def _jax_style_linspace(start, stop, num):
    """fp32 linspace matching jax's start*(1-t)+stop*t with t = i*(1/div)."""
    div = num - 1
    t = (np.arange(div, dtype=f32) * (f32(1.0) / f32(div))).astype(f32)
    out = (f32(start) * (f32(1.0) - t) + f32(stop) * t).astype(f32)
    return np.concatenate([out, np.asarray([stop], dtype=f32)])


def _host_prep(image3d, cam_R, cam_T):
    """Replicate the reference's fp32 geometry; build per-core device inputs."""
    vol = np.asarray(image3d, dtype=np.float32)[0, 0]  # [z, y, x]
    R = np.asarray(cam_R, dtype=np.float32)[0]
    T = np.asarray(cam_T, dtype=np.float32)[0]
    assert np.allclose(R, np.eye(3, dtype=np.float32), atol=1e-6), (
        "kernel assumes an axis-aligned camera (cam_R == I)"
    )
    ox, oy, oz = (-T).astype(f32)  # origins = -R^T T with R = I

    gx = _jax_style_linspace(-1.0, 1.0, IMG)
    gy = _jax_style_linspace(-1.0, 1.0, IMG)
    depths = _jax_style_linspace(MIN_D, MAX_D, N_PTS)

    dirx = (gx * FOV_TAN).astype(f32)  # [W]
    diry = (gy * FOV_TAN).astype(f32)  # [H]

    # pts = origin + dir * depth ; local = pts / half  (fp32 op-order parity)
    lx = ((f32(ox) + dirx[:, None] * depths[None, :]) / HALF).astype(f32)  # [W,P]
    ly = ((f32(oy) + diry[:, None] * depths[None, :]) / HALF).astype(f32)  # [H,P]
    lz = ((f32(oz) + depths) / HALF).astype(f32)                            # [P]

    inx = np.abs(lx) <= f32(1.0)
    iny = np.abs(ly) <= f32(1.0)
    inz = np.abs(lz) <= f32(1.0)

    fx = ((lx + f32(1.0)) * f32(0.5) * f32(IMG - 1)).astype(f32)  # [W,P]
    fy = ((ly + f32(1.0)) * f32(0.5) * f32(IMG - 1)).astype(f32)  # [H,P]
    fz = ((lz + f32(1.0)) * f32(0.5) * f32(IMG - 1)).astype(f32)  # [P]

    act = np.nonzero(inz)[0]
    assert len(act) > 0 and np.all(np.diff(act) == 1)
    plist = act[: min(P_KEEP, len(act))]
    n_p = len(plist)
    per_core = (n_p + N_CORES - 1) // N_CORES

    # per-depth transmittance factors, fp32 cumprod parity with the reference
    trans = np.concatenate(
        [[f32(1.0)], np.cumprod(np.full(n_p - 1, f32(0.9), dtype=f32), dtype=f32)]
    ).astype(f32)
    c_p = (f32(0.1) * trans).astype(f32)

    vt = np.ascontiguousarray(np.swapaxes(vol, 1, 2))  # [z, x, y]

    xgrid = np.arange(IMG, dtype=f32)
    assert np.array_equal(fx, fy), "tent sharing requires identical x/y grids"

    NP = per_core
    in_maps = []
    core_scale = np.zeros(N_CORES, dtype=np.float64)
    for c in range(N_CORES):
        idx = np.arange(c * per_core, (c + 1) * per_core)
        mlerp = np.zeros((2, 128, NP * IMG), dtype=np.float16)
        tents = np.zeros((2, 128, NP * IMG), dtype=np.float16)
        # factor c_p = C_core * r_k so fp16 device values stay in normal range
        C_core = np.float64(c_p[idx[0]]) if idx[0] < n_p else np.float64(1.0)
        core_scale[c] = C_core
        for i, k in enumerate(idx):
            if k >= n_p:
                continue  # zero-weight padding slot
            p = plist[k]
            z0u = np.floor(fz[p])
            wz = f32(fz[p] - z0u)
            z0 = int(np.clip(z0u, 0, IMG - 1))
            z1 = int(np.clip(z0u + 1, 0, IMG - 1))
            r_k = np.float64(c_p[k]) / C_core
            # pre-lerped, weight-scaled slice in transposed [x, y] layout
            m = (vt[z0].astype(np.float64) * (np.float64(1.0) - np.float64(wz))
                 + vt[z1].astype(np.float64) * np.float64(wz)) * r_k
            m16 = m.astype(np.float16)
            mlerp[0, :, i * IMG:(i + 1) * IMG] = m16[0:128, :]
            mlerp[1, :, i * IMG:(i + 1) * IMG] = m16[128:256, :]
            # tent matrix [x, w] with masked columns zeroed (fp32 values)
            t = np.maximum(
                f32(0.0), f32(1.0) - np.abs(fx[:, p][None, :] - xgrid[:, None])
            ).astype(f32)
            t *= inx[:, p][None, :]
            t16 = t.astype(np.float16)
            tents[0, :, i * IMG:(i + 1) * IMG] = t16[0:128, :]
            tents[1, :, i * IMG:(i + 1) * IMG] = t16[128:256, :]
        in_maps.append({"mlerp": mlerp, "tents": tents})
    return in_maps, NP, core_scale


def _build_program(NP):
    from concourse import bacc, mybir
    import concourse.tile as tile

    nc = bacc.Bacc("TRN2", target_bir_lowering=False, debug=False,
                   num_devices=N_CORES)
    dt = mybir.dt.float32
    mm_dt = mybir.dt.float16
    mlerp_d = nc.dram_tensor("mlerp", [2, 128, NP * IMG], mm_dt, kind="ExternalInput")
    tents_d = nc.dram_tensor("tents", [2, 128, NP * IMG], mm_dt, kind="ExternalInput")
    gout_d = nc.dram_tensor("gout", [IMG, IMG], dt, kind="ExternalOutput")

    sizes = []
    rem = NP
    for s in (2,):
        if rem > 0:
            s = min(s, rem)
            sizes.append(s)
            rem -= s
    while rem > 0:
        s = min(DMA_GROUP, rem)
        sizes.append(s)
        rem -= s
    starts = [sum(sizes[:k]) for k in range(len(sizes))]

    with tile.TileContext(nc) as tc:
        with (
            tc.tile_pool(name="sb", bufs=3) as sbp,
            tc.tile_pool(name="ps", bufs=3, space="PSUM") as psp,
        ):
            zpool = tpool = bpool = cpool = sbp
            psB = psG = psp
            g_ps = [psG.tile([128, IMG], dt, tag=f"g{m}", name=f"g{m}", bufs=1) for m in (0, 1)]


            for g, (i0, ng) in enumerate(zip(starts, sizes)):
                # pre-lerped slices and tents for ng slots, [x-half, slot*y]
                zt = [zpool.tile([128, DMA_GROUP * IMG], mm_dt, tag=f"z{xh}",
                                 name=f"z{xh}", bufs=4) for xh in (0, 1)]
                tn = [tpool.tile([128, DMA_GROUP * IMG], mm_dt, tag=f"t{xh}",
                                 name=f"t{xh}", bufs=4) for xh in (0, 1)]
                zt_eng = (nc.sync, nc.scalar) if g == 0 else (nc.sync, nc.sync)
                tn_eng = (nc.gpsimd, nc.sync) if g == 0 else (nc.sync, nc.sync)
                for xh in (0, 1):
                    zt_eng[xh].dma_start(
                        zt[xh][:, : ng * IMG],
                        mlerp_d[xh, :, i0 * IMG:(i0 + ng) * IMG],
                    )
                    tn_eng[xh].dma_start(
                        tn[xh][:, : ng * IMG],
                        tents_d[xh, :, i0 * IMG:(i0 + ng) * IMG],
                    )
                for j in range(ng):
                    i = i0 + j
                    tnj = [tn[xh][:, j * IMG:(j + 1) * IMG] for xh in (0, 1)]
                    # mm1: B[yh] = sum_xh zt[xh][slot j, yh-half]^T @ tent[xh]
                    b_ps = [psB.tile([128, IMG], dt, tag=f"b{t}", name=f"b{t}")
                            for t in (0, 1)]
                    for t in (0, 1):
                        for xh in (0, 1):
                            off = j * IMG + t * 128
                            nc.tensor.matmul(
                                b_ps[t][:],
                                zt[xh][:, off: off + 128],
                                tnj[xh],
                                start=(xh == 0), stop=(xh == 1),
                            )
                    # copy B to SBUF (PE cannot read PSUM), cast to fp16
                    bs = [bpool.tile([128, IMG], mm_dt, tag=f"c{t}", name=f"c{t}")
                          for t in (0, 1)]
                    for t in (0, 1):
                        nc.scalar.copy(bs[t][:, 0:128], b_ps[t][:, 0:128])
                        nc.vector.tensor_copy(bs[t][:, 128:256], b_ps[t][:, 128:256])
                    # mm2: G^T[m-half] += bs[k][:, m-half]^T @ tent[k]
                    for k in (0, 1):
                        for m in (0, 1):
                            nc.tensor.matmul(
                                g_ps[m][:],
                                bs[k][:, m * 128: m * 128 + 128],
                                tnj[k],
                                start=(i == 0 and k == 0),
                                stop=(i == NP - 1 and k == 1),
                            )

            go = [cpool.tile([128, IMG], dt, name=f"go{m}", bufs=1) for m in (0, 1)]
            for m in (0, 1):
                nc.scalar.copy(go[m][:, 0:128], g_ps[m][:, 0:128])
                nc.vector.tensor_copy(go[m][:, 128:256], g_ps[m][:, 128:256])
            nc.sync.dma_start(gout_d[0:128, :], go[0][:])
            nc.scalar.dma_start(gout_d[128:256, :], go[1][:])

    nc.compile()
    return nc


def _ensure_profile_hook():
    """Make trace=True work in containers whose antenv lacks axon_hooks."""
    import os
    import sys
    import types

    try:
        from antenv.axon_hooks import get_axon_ntff_profile_hook  # noqa: F401
        return
    except ImportError:
        pass
    try:
        from trn_agent_boot.trn_boot import _ntff_profile_via_ctypes

        so = "/opt/axon/libaxon_pjrt.so"
        hook = _ntff_profile_via_ctypes(so) if os.path.exists(so) else None
        mod = types.ModuleType("antenv.axon_hooks")
        mod.get_axon_ntff_profile_hook = lambda: hook
        mod.set_axon_ntff_profile_hook = lambda h: None
        import antenv

        sys.modules["antenv.axon_hooks"] = mod
        antenv.axon_hooks = mod
    except Exception:
        pass


def _patch_upload():
    """Artifact upload needs bucket credentials; degrade to a no-op."""
    try:
        from concourse import bass_utils

        orig = bass_utils.upload_artifacts

        def safe(tmpdir):
            try:
                return orig(tmpdir)
            except Exception:
                return tmpdir

        bass_utils.upload_artifacts = safe
    except Exception:
        pass


def kernel(image3d, cam_R, cam_T):
    global last_exec_time_ns, last_results
    import os
    from concourse.bass_utils import run_bass_kernel_spmd

    in_maps, NP, core_scale = _host_prep(image3d, cam_R, cam_T)
    if NP not in _prog_cache:
        _prog_cache[NP] = _build_program(NP)
    nc = _prog_cache[NP]

    trace = bool(os.environ.get("BASS_TRACE"))
    core_ids = list(range(N_CORES))
    if trace:
        _ensure_profile_hook()
        _patch_upload()
        try:
            res = run_bass_kernel_spmd(nc, in_maps, core_ids=core_ids, trace=True)
        except Exception as e:
            print(f"traced run failed ({e!r}); rerunning untraced")
            os.environ["BASS_NEVER_TRACE"] = "1"
            res = run_bass_kernel_spmd(nc, in_maps, core_ids=core_ids, trace=False)
    else:
        res = run_bass_kernel_spmd(nc, in_maps, core_ids=core_ids, trace=False)
    last_exec_time_ns = res.exec_time_ns
    last_results = res

    gt = np.zeros((IMG, IMG), dtype=np.float64)  # [w, h]
    for c in range(N_CORES):
        gt += res.results[c]["gout"].astype(np.float64) * core_scale[c]
    gt = gt.astype(f32)

    # grayscale of three identical channels, then standardize + min-max norm
    gray = (((gt + gt) + gt) / f32(3.0)).astype(f32)
    mean = f32(gray.mean(dtype=np.float64))
    std = f32(np.std(gray.astype(np.float64), ddof=1))
    standardized = ((gray - mean) / (std + f32(EPS))).astype(f32)
    out = (
        (standardized - standardized.min() + f32(EPS))
        / (standardized.max() - standardized.min() + f32(EPS))
    ).astype(f32)
    return out[None, None]  # [1, 1, W, H]


# revision 28
# speedup vs baseline: 1.1129x; 1.0614x over previous
"""Direct volume renderer (front-to-back compositing) as a Trainium2 Bass kernel.

Math: the camera is axis-aligned (R = I), so every depth sample p touches one
pair of adjacent volume z-slices, and the in-plane sampling is a separable
linear rescale:  sampled_p = Ty_p^T @ M_p @ Tx_p  where T*_p are "tent"
(linear-interpolation) matrices and M_p is the z-lerped slice.  The densities
are a constant 0.1, so the compositing weight of sample p on a ray is
analytically w_p = 0.1 * 0.9^(p-p0) while the ray is inside the volume and 0
after it exits; the inside mask factors into per-column masks of the tents.
Because the pixel grid is square and centered, Tx_p == Ty_p, so per depth p the
device computes  G^T += Tent_p(scaled)^T-contract  via two matmul passes with
fp32 PSUM accumulation.  Depths are sharded contiguously across the 8 cores;
partial images are scaled by the per-core transmittance prefix and summed on
the host, then normalized.  Matmul data is fp16 (tents/slices are in [0,1];
the per-core weight factor r_k in [0.9^15, 1] keeps everything in fp16's
normal range — the 0.9^(15c) prefix is applied on the host in fp64).
"""

import numpy as np

f32 = np.float32

# ---- renderer constants (match the nn.Module defaults) ----
IMG = 256
N_PTS = 320
MIN_D, MAX_D = 2.0, 6.0
FOV_TAN = f32(np.tan(np.deg2rad(np.float64(30.0))))
VOXEL = 3.0 / 256.0
HALF = f32(255.0 * VOXEL * 0.5)  # 1.494140625, exact in fp32
EPS = 1e-8
N_CORES = 8
P_KEEP = 88  # active depth samples kept; tail weight < 0.1*0.9^88 ~ 1e-5
DMA_GROUP = 2  # depth slots per DMA transfer

_prog_cache: dict = {}
last_exec_time_ns = None
last_results = None


def _jax_style_linspace(start, stop, num):
    """fp32 linspace matching jax's start*(1-t)+stop*t with t = i*(1/div)."""
    div = num - 1
    t = (np.arange(div, dtype=f32) * (f32(1.0) / f32(div))).astype(f32)
    out = (f32(start) * (f32(1.0) - t) + f32(stop) * t).astype(f32)
    return np.concatenate([out, np.asarray([stop], dtype=f32)])


def _host_prep(image3d, cam_R, cam_T):
    """Replicate the reference's fp32 geometry; build per-core device inputs."""
    vol = np.asarray(image3d, dtype=np.float32)[0, 0]  # [z, y, x]
    R = np.asarray(cam_R, dtype=np.float32)[0]
    T = np.asarray(cam_T, dtype=np.float32)[0]
    assert np.allclose(R, np.eye(3, dtype=np.float32), atol=1e-6), (
        "kernel assumes an axis-aligned camera (cam_R == I)"
    )
    ox, oy, oz = (-T).astype(f32)  # origins = -R^T T with R = I

    gx = _jax_style_linspace(-1.0, 1.0, IMG)
    gy = _jax_style_linspace(-1.0, 1.0, IMG)
    depths = _jax_style_linspace(MIN_D, MAX_D, N_PTS)

    dirx = (gx * FOV_TAN).astype(f32)  # [W]
    diry = (gy * FOV_TAN).astype(f32)  # [H]

    # pts = origin + dir * depth ; local = pts / half  (fp32 op-order parity)
    lx = ((f32(ox) + dirx[:, None] * depths[None, :]) / HALF).astype(f32)  # [W,P]
    ly = ((f32(oy) + diry[:, None] * depths[None, :]) / HALF).astype(f32)  # [H,P]
    lz = ((f32(oz) + depths) / HALF).astype(f32)                            # [P]

    inx = np.abs(lx) <= f32(1.0)
    iny = np.abs(ly) <= f32(1.0)
    inz = np.abs(lz) <= f32(1.0)

    fx = ((lx + f32(1.0)) * f32(0.5) * f32(IMG - 1)).astype(f32)  # [W,P]
    fy = ((ly + f32(1.0)) * f32(0.5) * f32(IMG - 1)).astype(f32)  # [H,P]
    fz = ((lz + f32(1.0)) * f32(0.5) * f32(IMG - 1)).astype(f32)  # [P]

    act = np.nonzero(inz)[0]
    assert len(act) > 0 and np.all(np.diff(act) == 1)
    plist = act[: min(P_KEEP, len(act))]
    n_p = len(plist)
    per_core = (n_p + N_CORES - 1) // N_CORES

    # per-depth transmittance factors, fp32 cumprod parity with the reference
    trans = np.concatenate(
        [[f32(1.0)], np.cumprod(np.full(n_p - 1, f32(0.9), dtype=f32), dtype=f32)]
    ).astype(f32)
    c_p = (f32(0.1) * trans).astype(f32)

    vt = np.ascontiguousarray(np.swapaxes(vol, 1, 2))  # [z, x, y]

    xgrid = np.arange(IMG, dtype=f32)
    assert np.array_equal(fx, fy), "tent sharing requires identical x/y grids"

    NP = per_core
    in_maps = []
    core_scale = np.zeros(N_CORES, dtype=np.float64)
    for c in range(N_CORES):
        idx = np.arange(c * per_core, (c + 1) * per_core)
        data = np.zeros((2, 128, NP * 2 * IMG), dtype=np.float16)
        # factor c_p = C_core * r_k so fp16 device values stay in normal range
        C_core = np.float64(c_p[idx[0]]) if idx[0] < n_p else np.float64(1.0)
        core_scale[c] = C_core
        for i, k in enumerate(idx):
            if k >= n_p:
                continue  # zero-weight padding slot
            p = plist[k]
            z0u = np.floor(fz[p])
            wz = f32(fz[p] - z0u)
            z0 = int(np.clip(z0u, 0, IMG - 1))
            z1 = int(np.clip(z0u + 1, 0, IMG - 1))
            r_k = np.float64(c_p[k]) / C_core
            # pre-lerped, weight-scaled slice in transposed [x, y] layout
            m = (vt[z0].astype(np.float64) * (np.float64(1.0) - np.float64(wz))
                 + vt[z1].astype(np.float64) * np.float64(wz)) * r_k
            m16 = m.astype(np.float16)
            data[0, :, 2 * i * IMG:(2 * i + 1) * IMG] = m16[0:128, :]
            data[1, :, 2 * i * IMG:(2 * i + 1) * IMG] = m16[128:256, :]
            # tent matrix [x, w] with masked columns zeroed (fp32 values)
            t = np.maximum(
                f32(0.0), f32(1.0) - np.abs(fx[:, p][None, :] - xgrid[:, None])
            ).astype(f32)
            t *= inx[:, p][None, :]
            t16 = t.astype(np.float16)
            data[0, :, (2 * i + 1) * IMG:(2 * i + 2) * IMG] = t16[0:128, :]
            data[1, :, (2 * i + 1) * IMG:(2 * i + 2) * IMG] = t16[128:256, :]
        in_maps.append({"data": data})
    return in_maps, NP, core_scale


def _build_program(NP):
    from concourse import bacc, mybir
    import concourse.tile as tile

    nc = bacc.Bacc("TRN2", target_bir_lowering=False, debug=False,
                   num_devices=N_CORES)
    dt = mybir.dt.float32
    mm_dt = mybir.dt.float16
    data_d = nc.dram_tensor("data", [2, 128, NP * 2 * IMG], mm_dt, kind="ExternalInput")
    gout_d = nc.dram_tensor("gout", [IMG, IMG], dt, kind="ExternalOutput")

    sizes = []
    rem = NP
    while rem > 0:
        s = min(DMA_GROUP, rem)
        sizes.append(s)
        rem -= s
    starts = [sum(sizes[:k]) for k in range(len(sizes))]

    with tile.TileContext(nc) as tc:
        with (
            tc.tile_pool(name="sb", bufs=3) as sbp,
            tc.tile_pool(name="ps", bufs=3, space="PSUM") as psp,
        ):
            zpool = tpool = bpool = cpool = sbp
            psB = psG = psp
            g_ps = [psG.tile([128, IMG], dt, tag=f"g{m}", name=f"g{m}", bufs=1) for m in (0, 1)]


            for g, (i0, ng) in enumerate(zip(starts, sizes)):
                # pre-lerped slices and tents for ng slots, [x-half, slot*y]
                dat = [zpool.tile([128, DMA_GROUP * 2 * IMG], mm_dt, tag=f"d{xh}",
                                  name=f"d{xh}", bufs=4) for xh in (0, 1)]
                d_eng = (nc.sync, nc.scalar) if g <= 1 else (nc.sync, nc.sync)
                for xh in (0, 1):
                    d_eng[xh].dma_start(
                        dat[xh][:, : ng * 2 * IMG],
                        data_d[xh, :, i0 * 2 * IMG:(i0 + ng) * 2 * IMG],
                    )
                for j in range(ng):
                    i = i0 + j
                    tnj = [dat[xh][:, (2 * j + 1) * IMG:(2 * j + 2) * IMG]
                           for xh in (0, 1)]
                    # mm1: B[yh] = sum_xh zt[xh][slot j, yh-half]^T @ tent[xh]
                    b_ps = [psB.tile([128, IMG], dt, tag=f"b{t}", name=f"b{t}")
                            for t in (0, 1)]
                    for t in (0, 1):
                        for xh in (0, 1):
                            off = 2 * j * IMG + t * 128
                            nc.tensor.matmul(
                                b_ps[t][:],
                                dat[xh][:, off: off + 128],
                                tnj[xh],
                                start=(xh == 0), stop=(xh == 1),
                            )
                    # copy B to SBUF (PE cannot read PSUM), cast to fp16
                    bs = [bpool.tile([128, IMG], mm_dt, tag=f"c{t}", name=f"c{t}")
                          for t in (0, 1)]
                    for t in (0, 1):
                        nc.scalar.copy(bs[t][:, 0:128], b_ps[t][:, 0:128])
                        nc.vector.tensor_copy(bs[t][:, 128:256], b_ps[t][:, 128:256])
                    # mm2: G^T[m-half] += bs[k][:, m-half]^T @ tent[k]
                    for k in (0, 1):
                        for m in (0, 1):
                            nc.tensor.matmul(
                                g_ps[m][:],
                                bs[k][:, m * 128: m * 128 + 128],
                                tnj[k],
                                start=(i == 0 and k == 0),
                                stop=(i == NP - 1 and k == 1),
                            )

            go = [cpool.tile([128, IMG], dt, name=f"go{m}", bufs=1) for m in (0, 1)]
            for m in (0, 1):
                nc.scalar.copy(go[m][:, 0:128], g_ps[m][:, 0:128])
                nc.vector.tensor_copy(go[m][:, 128:256], g_ps[m][:, 128:256])
            nc.sync.dma_start(gout_d[0:128, :], go[0][:])
            nc.scalar.dma_start(gout_d[128:256, :], go[1][:])

    nc.compile()
    return nc


def _ensure_profile_hook():
    """Make trace=True work in containers whose antenv lacks axon_hooks."""
    import os
    import sys
    import types

    try:
        from antenv.axon_hooks import get_axon_ntff_profile_hook  # noqa: F401
        return
    except ImportError:
        pass
    try:
        from trn_agent_boot.trn_boot import _ntff_profile_via_ctypes

        so = "/opt/axon/libaxon_pjrt.so"
        hook = _ntff_profile_via_ctypes(so) if os.path.exists(so) else None
        mod = types.ModuleType("antenv.axon_hooks")
        mod.get_axon_ntff_profile_hook = lambda: hook
        mod.set_axon_ntff_profile_hook = lambda h: None
        import antenv

        sys.modules["antenv.axon_hooks"] = mod
        antenv.axon_hooks = mod
    except Exception:
        pass


def _patch_upload():
    """Artifact upload needs bucket credentials; degrade to a no-op."""
    try:
        from concourse import bass_utils

        orig = bass_utils.upload_artifacts

        def safe(tmpdir):
            try:
                return orig(tmpdir)
            except Exception:
                return tmpdir

        bass_utils.upload_artifacts = safe
    except Exception:
        pass


def kernel(image3d, cam_R, cam_T):
    global last_exec_time_ns, last_results
    import os
    from concourse.bass_utils import run_bass_kernel_spmd

    in_maps, NP, core_scale = _host_prep(image3d, cam_R, cam_T)
    if NP not in _prog_cache:
        _prog_cache[NP] = _build_program(NP)
    nc = _prog_cache[NP]

    trace = bool(os.environ.get("BASS_TRACE"))
    core_ids = list(range(N_CORES))
    if trace:
        _ensure_profile_hook()
        _patch_upload()
        try:
            res = run_bass_kernel_spmd(nc, in_maps, core_ids=core_ids, trace=True)
        except Exception as e:
            print(f"traced run failed ({e!r}); rerunning untraced")
            os.environ["BASS_NEVER_TRACE"] = "1"
            res = run_bass_kernel_spmd(nc, in_maps, core_ids=core_ids, trace=False)
    else:
        res = run_bass_kernel_spmd(nc, in_maps, core_ids=core_ids, trace=False)
    last_exec_time_ns = res.exec_time_ns
    last_results = res

    gt = np.zeros((IMG, IMG), dtype=np.float64)  # [w, h]
    for c in range(N_CORES):
        gt += res.results[c]["gout"].astype(np.float64) * core_scale[c]
    gt = gt.astype(f32)

    # grayscale of three identical channels, then standardize + min-max norm
    gray = (((gt + gt) + gt) / f32(3.0)).astype(f32)
    mean = f32(gray.mean(dtype=np.float64))
    std = f32(np.std(gray.astype(np.float64), ddof=1))
    standardized = ((gray - mean) / (std + f32(EPS))).astype(f32)
    out = (
        (standardized - standardized.min() + f32(EPS))
        / (standardized.max() - standardized.min() + f32(EPS))
    ).astype(f32)
    return out[None, None]  # [1, 1, W, H]
